# revision 2
# baseline (speedup 1.0000x reference)
"""Bidirectional cross-attention Trainium2 kernel.

Data-parallel over batch B=8 across 8 NeuronCores (1 sample/core).

Per-core dataflow (all matmuls f32r):
  Q1[c,n], K2[c,n]   : 1x1-conv projections, bias added per-partition on DVE
  V2T[n,c]+bias, plus a ones column at c=256 (bias folded via K=1 matmul)
  expS1T[j,i] = exp(K2^T Q1)  (ScalarE exp, chunked over i; no max-subtract,
                               logits are bounded for this problem's scale)
  outT[i, 0:257] = sum_j expS1T[j,i] * [V2T+b | 1][j, :]
     -> col 256 is the softmax denominator; y = outT[:,0:256]/denominator + x1T
  (symmetric for direction 2)

Host side: shard batch, transpose weights/x, gather + transpose outputs.
"""

import sys

if "/opt/trn_rl_repo" not in sys.path:
    sys.path.insert(0, "/opt/trn_rl_repo")

import numpy as np

B, C, H, W = 8, 256, 48, 48
N = H * W  # 2304
NT = N // 128  # 18 j/i tiles
CT = C // 128  # 2 c tiles
CW = 384  # i-chunk width for expS
NCH = N // CW  # 6 chunks
ISUB = CW // 128  # 3 i-subtiles per chunk

_CACHE = {}


def _build():
    import concourse.bacc as bacc
    import concourse.mybir as mybir
    from concourse.tile import TileContext

    F32, F32R = mybir.dt.float32, mybir.dt.float32r
    Exp = mybir.ActivationFunctionType.Exp

    nc = bacc.Bacc(None, target_bir_lowering=False)

    x_d = {
        "x1": nc.dram_tensor("x1", [C, N], F32R, kind="ExternalInput"),
        "x2": nc.dram_tensor("x2", [C, N], F32R, kind="ExternalInput"),
    }
    xt_d = {
        "x1t": nc.dram_tensor("x1t", [N, C], F32, kind="ExternalInput"),
        "x2t": nc.dram_tensor("x2t", [N, C], F32, kind="ExternalInput"),
    }
    w_names = ["wq1t", "wk1t", "wv1t", "wq2t", "wk2t", "wv2t"]
    w_d = {n: nc.dram_tensor(n, [C, C], F32R, kind="ExternalInput") for n in w_names}
    bqk_names = ["bq1", "bk1", "bq2", "bk2"]
    bqk_d = {n: nc.dram_tensor(n, [C, 1], F32, kind="ExternalInput") for n in bqk_names}
    bv_d = {
        n: nc.dram_tensor(n, [1, C + 2], F32R, kind="ExternalInput") for n in ["bv1", "bv2"]
    }
    ones_d = nc.dram_tensor("ones", [1, 128], F32R, kind="ExternalInput")
    y_d = {
        "y1t": nc.dram_tensor("y1t", [N, C], F32, kind="ExternalOutput"),
        "y2t": nc.dram_tensor("y2t", [N, C], F32, kind="ExternalOutput"),
    }

    with TileContext(nc) as tc:
        with (
            tc.tile_pool(name="const", bufs=1) as cp,
            tc.tile_pool(name="proj", bufs=1) as pp,
            tc.tile_pool(name="stream", bufs=4) as sp,
            tc.tile_pool(name="psum", bufs=2, space="PSUM") as psp,
        ):
            w_sb = {}
            for n in w_names:
                t = cp.tile([128, CT, C], F32R, tag=n)
                for ck in range(CT):
                    nc.sync.dma_start(out=t[:, ck, :], in_=w_d[n][ck * 128 : (ck + 1) * 128, :])
                w_sb[n] = t
            bqk_sb = {}
            for n in bqk_names:
                t = cp.tile([128, CT], F32, tag=n)
                for ct in range(CT):
                    nc.sync.dma_start(
                        out=t[:, ct : ct + 1], in_=bqk_d[n][ct * 128 : (ct + 1) * 128, :]
                    )
                bqk_sb[n] = t
            bv_sb = {}
            for n in ["bv1", "bv2"]:
                t = cp.tile([1, C + 2], F32R, tag=n)
                nc.sync.dma_start(out=t[:, :], in_=bv_d[n][:, :])
                bv_sb[n] = t
            ones_sb = cp.tile([1, 128], F32R, tag="ones")
            nc.sync.dma_start(out=ones_sb[:, :], in_=ones_d[:, :])

            # ---------- projections (x tiles freed after this block) ----------
            proj = {}
            with tc.tile_pool(name="xp", bufs=1) as xp:
                x_sb = {}
                for n in ["x1", "x2"]:
                    t = xp.tile([128, CT, N], F32R, tag=n)
                    for ck in range(CT):
                        nc.sync.dma_start(
                            out=t[:, ck, :], in_=x_d[n][ck * 128 : (ck + 1) * 128, :]
                        )
                    x_sb[n] = t

                def proj_qk(dst, xt, wn, bn):
                    for ct in range(CT):
                        for ch in range(NCH):
                            ps = psp.tile([128, CW], F32, tag="pp")
                            for ck in range(CT):
                                nc.tensor.matmul(
                                    ps[:, :],
                                    w_sb[wn][:, ck, ct * 128 : (ct + 1) * 128],
                                    xt[:, ck, ch * CW : (ch + 1) * CW],
                                    start=(ck == 0),
                                    stop=(ck == CT - 1),
                                )
                            nc.vector.tensor_scalar_add(
                                dst[:, ct, ch * CW : (ch + 1) * CW],
                                ps[:, :],
                                bqk_sb[bn][:, ct : ct + 1],
                            )

                def proj_vt(dst, xt, wn, bn):
                    for jt in range(NT):
                        ps = psp.tile([128, C + 2], F32, tag="pv")
                        for ck in range(CT):
                            nc.tensor.matmul(
                                ps[:, 0:C],
                                xt[:, ck, jt * 128 : (jt + 1) * 128],
                                w_sb[wn][:, ck, :],
                                start=(ck == 0),
                                stop=False,
                                skip_group_check=True,
                            )
                        nc.tensor.matmul(
                            ps[:, :],
                            ones_sb[:, :],
                            bv_sb[bn][:, :],
                            start=False,
                            stop=True,
                            skip_group_check=True,
                        )
                        nc.vector.tensor_copy(dst[:, jt, :], ps[:, :])

                for nm, xn, wn, bn in [
                    ("Q1", "x1", "wq1t", "bq1"),
                    ("K2", "x2", "wk2t", "bk2"),
                    ("Q2", "x2", "wq2t", "bq2"),
                    ("K1", "x1", "wk1t", "bk1"),
                ]:
                    t = pp.tile([128, CT, N], F32R, tag=nm)
                    proj_qk(t, x_sb[xn], wn, bn)
                    proj[nm] = t
                for nm, xn, wn, bn in [
                    ("VT2", "x2", "wv2t", "bv2"),
                    ("VT1", "x1", "wv1t", "bv1"),
                ]:
                    t = pp.tile([128, NT, C + 2], F32R, tag=nm)
                    proj_vt(t, x_sb[xn], wn, bn)
                    proj[nm] = t

            # ---------- attention, one direction at a time ----------
            with tc.tile_pool(name="ep", bufs=2) as ep:

                def attention(Q, K, VT, xt_dram, yt_dram):
                    for ch in range(NCH):
                        e = ep.tile([128, NT, CW], F32R, tag="e")
                        for jt in range(NT):
                            ps = psp.tile([128, CW], F32, tag="ps_s")
                            for ck in range(CT):
                                nc.tensor.matmul(
                                    ps[:, :],
                                    K[:, ck, jt * 128 : (jt + 1) * 128],
                                    Q[:, ck, ch * CW : (ch + 1) * CW],
                                    start=(ck == 0),
                                    stop=(ck == CT - 1),
                                )
                            nc.scalar.activation(e[:, jt, :], ps[:, :], Exp)
                        for il in range(ISUB):
                            it = ch * ISUB + il
                            po = psp.tile([128, C + 2], F32, tag="ps_o")
                            for jt in range(NT):
                                nc.tensor.matmul(
                                    po[:, :],
                                    e[:, jt, il * 128 : (il + 1) * 128],
                                    VT[:, jt, :],
                                    start=(jt == 0),
                                    stop=(jt == NT - 1),
                                )
                            r = sp.tile([128, 1], F32, tag="r")
                            nc.vector.reciprocal(r[:, :], po[:, C : C + 1])
                            xt_t = sp.tile([128, C], F32, tag="xt")
                            nc.sync.dma_start(
                                out=xt_t[:, :], in_=xt_dram[it * 128 : (it + 1) * 128, :]
                            )
                            y = sp.tile([128, C], F32, tag="y")
                            nc.vector.tensor_scalar_mul(y[:, :], po[:, 0:C], r[:, :])
                            nc.vector.tensor_add(y[:, :], y[:, :], xt_t[:, :])
                            nc.sync.dma_start(
                                out=yt_dram[it * 128 : (it + 1) * 128, :], in_=y[:, :]
                            )

                attention(proj["Q1"], proj["K2"], proj["VT2"], xt_d["x1t"], y_d["y1t"])
                attention(proj["Q2"], proj["K1"], proj["VT1"], xt_d["x2t"], y_d["y2t"])

    nc.compile()
    return nc


def _get_nc():
    if "nc" not in _CACHE:
        _CACHE["nc"] = _build()
    return _CACHE["nc"]


def kernel(
    x1,
    x2,
    w_q1,
    b_q1,
    w_k1,
    b_k1,
    w_v1,
    b_v1,
    w_q2,
    b_q2,
    w_k2,
    b_k2,
    w_v2,
    b_v2,
    _trace=False,
):
    from concourse.bass_utils import run_bass_kernel_spmd

    nc = _get_nc()

    x1 = np.asarray(x1, dtype=np.float32)
    x2 = np.asarray(x2, dtype=np.float32)
    wT = {
        "wq1t": np.ascontiguousarray(np.asarray(w_q1, np.float32).T),
        "wk1t": np.ascontiguousarray(np.asarray(w_k1, np.float32).T),
        "wv1t": np.ascontiguousarray(np.asarray(w_v1, np.float32).T),
        "wq2t": np.ascontiguousarray(np.asarray(w_q2, np.float32).T),
        "wk2t": np.ascontiguousarray(np.asarray(w_k2, np.float32).T),
        "wv2t": np.ascontiguousarray(np.asarray(w_v2, np.float32).T),
    }
    bqk = {
        "bq1": np.asarray(b_q1, np.float32).reshape(C, 1),
        "bk1": np.asarray(b_k1, np.float32).reshape(C, 1),
        "bq2": np.asarray(b_q2, np.float32).reshape(C, 1),
        "bk2": np.asarray(b_k2, np.float32).reshape(C, 1),
    }
    bv = {
        "bv1": np.concatenate(
            [np.asarray(b_v1, np.float32).reshape(1, C), np.ones((1, 2), np.float32)], 1
        ),
        "bv2": np.concatenate(
            [np.asarray(b_v2, np.float32).reshape(1, C), np.ones((1, 2), np.float32)], 1
        ),
    }
    ones = np.ones((1, 128), np.float32)

    in_maps = []
    for i in range(B):
        x1i = np.ascontiguousarray(x1[i].reshape(C, N))
        x2i = np.ascontiguousarray(x2[i].reshape(C, N))
        m = {
            "x1": x1i,
            "x2": x2i,
            "x1t": np.ascontiguousarray(x1i.T),
            "x2t": np.ascontiguousarray(x2i.T),
            "ones": ones,
        }
        m.update(wT)
        m.update(bqk)
        m.update(bv)
        in_maps.append(m)

    res = run_bass_kernel_spmd(nc, in_maps, list(range(B)), trace=_trace)
    if _trace:
        _CACHE["last_result"] = res

    y1 = np.empty((B, C, H, W), np.float32)
    y2 = np.empty((B, C, H, W), np.float32)
    for i in range(B):
        y1[i] = res.results[i]["y1t"].T.reshape(C, H, W)
        y2[i] = res.results[i]["y2t"].T.reshape(C, H, W)
    return y1, y2


# revision 4
# speedup vs baseline: 1.0731x; 1.0731x over previous
"""Bidirectional cross-attention Trainium2 kernel.

Data-parallel over batch B=8 across 8 NeuronCores (1 sample/core).

Per-core dataflow (all matmuls f32r):
  Q1[c,n], K2[c,n]   : 1x1-conv projections, bias added per-partition on DVE
  V2T[n,c]+bias, plus a ones column at c=256 (bias folded via K=1 matmul)
  expS1T[j,i] = exp(K2^T Q1)  (ScalarE exp, chunked over i; no max-subtract,
                               logits are bounded for this problem's scale)
  outT[i, 0:257] = sum_j expS1T[j,i] * [V2T+b | 1][j, :]
     -> col 256 is the softmax denominator; y = outT[:,0:256]/denominator + x1T
  (symmetric for direction 2)

Host side: shard batch, transpose weights/x, gather + transpose outputs.
"""

import sys

if "/opt/trn_rl_repo" not in sys.path:
    sys.path.insert(0, "/opt/trn_rl_repo")

import numpy as np

B, C, H, W = 8, 256, 48, 48
N = H * W  # 2304
NT = N // 128  # 18 j/i tiles
CT = C // 128  # 2 c tiles
CW = 512  # max i-chunk width for expS (last chunk is 256)
CHUNKS = [(0, 512), (512, 512), (1024, 512), (1536, 512), (2048, 256)]

_CACHE = {}


def _build():
    import concourse.bacc as bacc
    import concourse.mybir as mybir
    from concourse.tile import TileContext

    F32, F32R, BF16 = mybir.dt.float32, mybir.dt.float32r, mybir.dt.bfloat16
    Exp = mybir.ActivationFunctionType.Exp

    nc = bacc.Bacc(None, target_bir_lowering=False)

    x_d = {
        "x1": nc.dram_tensor("x1", [C, N], F32R, kind="ExternalInput"),
        "x2": nc.dram_tensor("x2", [C, N], F32R, kind="ExternalInput"),
    }
    xt_d = {
        "x1t": nc.dram_tensor("x1t", [N, C], F32, kind="ExternalInput"),
        "x2t": nc.dram_tensor("x2t", [N, C], F32, kind="ExternalInput"),
    }
    w_names = ["wq1t", "wk1t", "wv1t", "wq2t", "wk2t", "wv2t"]
    w_d = {n: nc.dram_tensor(n, [C, C], F32R, kind="ExternalInput") for n in w_names}
    bqk_names = ["bq1", "bk1", "bq2", "bk2"]
    bqk_d = {n: nc.dram_tensor(n, [C, 1], F32, kind="ExternalInput") for n in bqk_names}
    bv_d = {
        n: nc.dram_tensor(n, [1, C + 2], F32R, kind="ExternalInput") for n in ["bv1", "bv2"]
    }
    ones_d = nc.dram_tensor("ones", [1, 128], F32R, kind="ExternalInput")
    y_d = {
        "y1t": nc.dram_tensor("y1t", [N, C], F32, kind="ExternalOutput"),
        "y2t": nc.dram_tensor("y2t", [N, C], F32, kind="ExternalOutput"),
    }

    with TileContext(nc) as tc:
        with (
            tc.tile_pool(name="const", bufs=1) as cp,
            tc.tile_pool(name="proj", bufs=1) as pp,
            tc.tile_pool(name="stream", bufs=4) as sp,
            tc.tile_pool(name="psum", bufs=2, space="PSUM") as psp,
        ):
            # ---------- projections (x tiles freed after this block) ----------
            proj = {}
            with tc.tile_pool(name="xp", bufs=1) as xp:
                # x DMAs first (critical path), split for earlier first-arrival
                x_sb = {}
                for n in ["x1", "x2"]:
                    t = xp.tile([128, CT, N], F32R, tag=n)
                    for ck in range(CT):
                        for h0, hw in [(0, 768), (768, 768), (1536, 768)]:
                            nc.sync.dma_start(
                                out=t[:, ck, h0 : h0 + hw],
                                in_=x_d[n][ck * 128 : (ck + 1) * 128, h0 : h0 + hw],
                            )
                    x_sb[n] = t
                w_sb = {}
                for n in w_names:
                    t = cp.tile([128, CT, C], F32R, tag=n)
                    for ck in range(CT):
                        nc.sync.dma_start(
                            out=t[:, ck, :], in_=w_d[n][ck * 128 : (ck + 1) * 128, :]
                        )
                    w_sb[n] = t
                bqk_sb = {}
                for n in bqk_names:
                    t = cp.tile([128, CT], F32, tag=n)
                    for ct in range(CT):
                        nc.sync.dma_start(
                            out=t[:, ct : ct + 1], in_=bqk_d[n][ct * 128 : (ct + 1) * 128, :]
                        )
                    bqk_sb[n] = t
                bv_sb = {}
                for n in ["bv1", "bv2"]:
                    t = cp.tile([1, C + 2], F32R, tag=n)
                    nc.sync.dma_start(out=t[:, :], in_=bv_d[n][:, :])
                    bv_sb[n] = t
                ones_sb = cp.tile([1, 128], F32R, tag="ones")
                nc.sync.dma_start(out=ones_sb[:, :], in_=ones_d[:, :])

                def proj_qk(dst, xt, wn, bn):
                    for ct in range(CT):
                        for c0, cw in CHUNKS:
                            ps = psp.tile([128, CW], F32, tag="pp")
                            for ck in range(CT):
                                nc.tensor.matmul(
                                    ps[:, 0:cw],
                                    w_sb[wn][:, ck, ct * 128 : (ct + 1) * 128],
                                    xt[:, ck, c0 : c0 + cw],
                                    start=(ck == 0),
                                    stop=(ck == CT - 1),
                                )
                            nc.vector.tensor_scalar_add(
                                dst[:, ct, c0 : c0 + cw],
                                ps[:, 0:cw],
                                bqk_sb[bn][:, ct : ct + 1],
                            )

                def proj_vt(dst, xt, wn, bn):
                    for jt in range(NT):
                        ps = psp.tile([128, C + 2], F32, tag="pv")
                        for ck in range(CT):
                            nc.tensor.matmul(
                                ps[:, 0:C],
                                xt[:, ck, jt * 128 : (jt + 1) * 128],
                                w_sb[wn][:, ck, :],
                                start=(ck == 0),
                                stop=False,
                                skip_group_check=True,
                            )
                        nc.tensor.matmul(
                            ps[:, :],
                            ones_sb[:, :],
                            bv_sb[bn][:, :],
                            start=False,
                            stop=True,
                            skip_group_check=True,
                        )
                        nc.vector.tensor_copy(dst[:, jt, :], ps[:, :])

                for nm, xn, wn, bn in [
                    ("Q1", "x1", "wq1t", "bq1"),
                    ("K2", "x2", "wk2t", "bk2"),
                    ("Q2", "x2", "wq2t", "bq2"),
                    ("K1", "x1", "wk1t", "bk1"),
                ]:
                    t = pp.tile([128, CT, N], F32R, tag=nm)
                    proj_qk(t, x_sb[xn], wn, bn)
                    proj[nm] = t
                for nm, xn, wn, bn in [
                    ("VT2", "x2", "wv2t", "bv2"),
                    ("VT1", "x1", "wv1t", "bv1"),
                ]:
                    t = pp.tile([128, NT, C + 2], BF16, tag=nm)
                    proj_vt(t, x_sb[xn], wn, bn)
                    proj[nm] = t

            # ---------- attention, one direction at a time ----------
            with tc.tile_pool(name="ep", bufs=2) as ep:

                def attention(Q, K, VT, xt_dram, yt_dram):
                    for c0, cw in CHUNKS:
                        e = ep.tile([128, NT, CW], BF16, tag="e")
                        for jt in range(NT):
                            ps = psp.tile([128, CW], F32, tag="ps_s")
                            for ck in range(CT):
                                nc.tensor.matmul(
                                    ps[:, 0:cw],
                                    K[:, ck, jt * 128 : (jt + 1) * 128],
                                    Q[:, ck, c0 : c0 + cw],
                                    start=(ck == 0),
                                    stop=(ck == CT - 1),
                                )
                            nc.scalar.activation(e[:, jt, 0:cw], ps[:, 0:cw], Exp)
                        for il in range(cw // 128):
                            it = c0 // 128 + il
                            po = psp.tile([128, C + 2], F32, tag="ps_o")
                            for jt in range(NT):
                                nc.tensor.matmul(
                                    po[:, :],
                                    e[:, jt, il * 128 : (il + 1) * 128],
                                    VT[:, jt, :],
                                    start=(jt == 0),
                                    stop=(jt == NT - 1),
                                )
                            r = sp.tile([128, 1], F32, tag="r")
                            nc.vector.reciprocal(r[:, :], po[:, C : C + 1])
                            xt_t = sp.tile([128, C], F32, tag="xt")
                            nc.sync.dma_start(
                                out=xt_t[:, :], in_=xt_dram[it * 128 : (it + 1) * 128, :]
                            )
                            y = sp.tile([128, C], F32, tag="y")
                            nc.vector.tensor_scalar_mul(y[:, :], po[:, 0:C], r[:, :])
                            nc.vector.tensor_add(y[:, :], y[:, :], xt_t[:, :])
                            nc.sync.dma_start(
                                out=yt_dram[it * 128 : (it + 1) * 128, :], in_=y[:, :]
                            )

                attention(proj["Q1"], proj["K2"], proj["VT2"], xt_d["x1t"], y_d["y1t"])
                attention(proj["Q2"], proj["K1"], proj["VT1"], xt_d["x2t"], y_d["y2t"])

    nc.compile()
    return nc


def _get_nc():
    if "nc" not in _CACHE:
        _CACHE["nc"] = _build()
    return _CACHE["nc"]


def kernel(
    x1,
    x2,
    w_q1,
    b_q1,
    w_k1,
    b_k1,
    w_v1,
    b_v1,
    w_q2,
    b_q2,
    w_k2,
    b_k2,
    w_v2,
    b_v2,
    _trace=False,
):
    from concourse.bass_utils import run_bass_kernel_spmd

    nc = _get_nc()

    x1 = np.asarray(x1, dtype=np.float32)
    x2 = np.asarray(x2, dtype=np.float32)
    wT = {
        "wq1t": np.ascontiguousarray(np.asarray(w_q1, np.float32).T),
        "wk1t": np.ascontiguousarray(np.asarray(w_k1, np.float32).T),
        "wv1t": np.ascontiguousarray(np.asarray(w_v1, np.float32).T),
        "wq2t": np.ascontiguousarray(np.asarray(w_q2, np.float32).T),
        "wk2t": np.ascontiguousarray(np.asarray(w_k2, np.float32).T),
        "wv2t": np.ascontiguousarray(np.asarray(w_v2, np.float32).T),
    }
    bqk = {
        "bq1": np.asarray(b_q1, np.float32).reshape(C, 1),
        "bk1": np.asarray(b_k1, np.float32).reshape(C, 1),
        "bq2": np.asarray(b_q2, np.float32).reshape(C, 1),
        "bk2": np.asarray(b_k2, np.float32).reshape(C, 1),
    }
    bv = {
        "bv1": np.concatenate(
            [np.asarray(b_v1, np.float32).reshape(1, C), np.ones((1, 2), np.float32)], 1
        ),
        "bv2": np.concatenate(
            [np.asarray(b_v2, np.float32).reshape(1, C), np.ones((1, 2), np.float32)], 1
        ),
    }
    ones = np.ones((1, 128), np.float32)

    in_maps = []
    for i in range(B):
        x1i = np.ascontiguousarray(x1[i].reshape(C, N))
        x2i = np.ascontiguousarray(x2[i].reshape(C, N))
        m = {
            "x1": x1i,
            "x2": x2i,
            "x1t": np.ascontiguousarray(x1i.T),
            "x2t": np.ascontiguousarray(x2i.T),
            "ones": ones,
        }
        m.update(wT)
        m.update(bqk)
        m.update(bv)
        in_maps.append(m)

    res = run_bass_kernel_spmd(nc, in_maps, list(range(B)), trace=_trace)
    if _trace:
        _CACHE["last_result"] = res

    y1 = np.empty((B, C, H, W), np.float32)
    y2 = np.empty((B, C, H, W), np.float32)
    for i in range(B):
        y1[i] = res.results[i]["y1t"].T.reshape(C, H, W)
        y2[i] = res.results[i]["y2t"].T.reshape(C, H, W)
    return y1, y2


# revision 5
# speedup vs baseline: 1.1104x; 1.0348x over previous
"""Bidirectional cross-attention Trainium2 kernel.

Data-parallel over batch B=8 across 8 NeuronCores (1 sample/core).

Per-core dataflow (all matmuls f32r):
  Q1[c,n], K2[c,n]   : 1x1-conv projections, bias added per-partition on DVE
  V2T[n,c]+bias, plus a ones column at c=256 (bias folded via K=1 matmul)
  expS1T[j,i] = exp(K2^T Q1)  (ScalarE exp, chunked over i; no max-subtract,
                               logits are bounded for this problem's scale)
  outT[i, 0:257] = sum_j expS1T[j,i] * [V2T+b | 1][j, :]
     -> col 256 is the softmax denominator; y = outT[:,0:256]/denominator + x1T
  (symmetric for direction 2)

Host side: shard batch, transpose weights/x, gather + transpose outputs.
"""

import sys

if "/opt/trn_rl_repo" not in sys.path:
    sys.path.insert(0, "/opt/trn_rl_repo")

import numpy as np

B, C, H, W = 8, 256, 48, 48
N = H * W  # 2304
NT = N // 128  # 18 j/i tiles
CT = C // 128  # 2 c tiles
CW = 512  # max i-chunk width for expS (last chunk is 256)
CHUNKS = [(0, 512), (512, 512), (1024, 512), (1536, 512), (2048, 256)]

_CACHE = {}


def _build():
    import concourse.bacc as bacc
    import concourse.mybir as mybir
    from concourse.tile import TileContext

    F32, F32R, BF16 = mybir.dt.float32, mybir.dt.float32r, mybir.dt.bfloat16
    Exp = mybir.ActivationFunctionType.Exp

    nc = bacc.Bacc(None, target_bir_lowering=False)

    x_d = {
        "x1": nc.dram_tensor("x1", [C, N], F32R, kind="ExternalInput"),
        "x2": nc.dram_tensor("x2", [C, N], F32R, kind="ExternalInput"),
    }
    xt_d = {
        "x1t": nc.dram_tensor("x1t", [N, C], F32, kind="ExternalInput"),
        "x2t": nc.dram_tensor("x2t", [N, C], F32, kind="ExternalInput"),
    }
    w_names = ["wq1t", "wk1t", "wv1t", "wq2t", "wk2t", "wv2t"]
    w_d = {n: nc.dram_tensor(n, [C, C], F32R, kind="ExternalInput") for n in w_names}
    bqk_names = ["bq1", "bk1", "bq2", "bk2"]
    bqk_d = {n: nc.dram_tensor(n, [C, 1], F32, kind="ExternalInput") for n in bqk_names}
    bv_d = {
        n: nc.dram_tensor(n, [1, C + 2], F32R, kind="ExternalInput") for n in ["bv1", "bv2"]
    }
    ones_d = nc.dram_tensor("ones", [1, 128], F32R, kind="ExternalInput")
    y_d = {
        "y1t": nc.dram_tensor("y1t", [N, C], F32, kind="ExternalOutput"),
        "y2t": nc.dram_tensor("y2t", [N, C], F32, kind="ExternalOutput"),
    }

    with TileContext(nc) as tc:
        with (
            tc.tile_pool(name="const", bufs=1) as cp,
            tc.tile_pool(name="proj", bufs=1) as pp,
            tc.tile_pool(name="stream", bufs=4) as sp,
            tc.tile_pool(name="psum", bufs=2, space="PSUM") as psp,
            tc.tile_pool(name="psum_s", bufs=4, space="PSUM") as psp2,
        ):
            # ---------- projections (x tiles freed after this block) ----------
            proj = {}
            with tc.tile_pool(name="xp", bufs=1) as xp:
                w_sb = {}

                def load_w(n):
                    t = cp.tile([128, CT, C], F32R, tag=n)
                    for ck in range(CT):
                        nc.sync.dma_start(
                            out=t[:, ck, :], in_=w_d[n][ck * 128 : (ck + 1) * 128, :]
                        )
                    w_sb[n] = t

                # first projection's weights before the bulk x transfers
                for n in ["wq1t", "wk2t", "wv2t"]:
                    load_w(n)
                x_sb = {}
                for n in ["x1", "x2"]:
                    t = xp.tile([128, CT, N], F32R, tag=n)
                    for ck in range(CT):
                        for h0, hw in [(0, 768), (768, 768), (1536, 768)]:
                            nc.sync.dma_start(
                                out=t[:, ck, h0 : h0 + hw],
                                in_=x_d[n][ck * 128 : (ck + 1) * 128, h0 : h0 + hw],
                            )
                    x_sb[n] = t
                for n in ["wq2t", "wk1t", "wv1t"]:
                    load_w(n)
                bqk_sb = {}
                for n in bqk_names:
                    t = cp.tile([128, CT], F32, tag=n)
                    for ct in range(CT):
                        nc.sync.dma_start(
                            out=t[:, ct : ct + 1], in_=bqk_d[n][ct * 128 : (ct + 1) * 128, :]
                        )
                    bqk_sb[n] = t
                bv_sb = {}
                for n in ["bv1", "bv2"]:
                    t = cp.tile([1, C + 2], F32R, tag=n)
                    nc.sync.dma_start(out=t[:, :], in_=bv_d[n][:, :])
                    bv_sb[n] = t
                ones_sb = cp.tile([1, 128], F32R, tag="ones")
                nc.sync.dma_start(out=ones_sb[:, :], in_=ones_d[:, :])

                def proj_qk(dst, xt, wn, bn):
                    for ct in range(CT):
                        for c0, cw in CHUNKS:
                            ps = psp.tile([128, CW], F32, tag="pj")
                            for ck in range(CT):
                                nc.tensor.matmul(
                                    ps[:, 0:cw],
                                    w_sb[wn][:, ck, ct * 128 : (ct + 1) * 128],
                                    xt[:, ck, c0 : c0 + cw],
                                    start=(ck == 0),
                                    stop=(ck == CT - 1),
                                )
                            nc.vector.tensor_scalar_add(
                                dst[:, ct, c0 : c0 + cw],
                                ps[:, 0:cw],
                                bqk_sb[bn][:, ct : ct + 1],
                            )

                def proj_vt(dst, xt, wn, bn):
                    for jt in range(NT):
                        ps = psp.tile([128, CW], F32, tag="pj")
                        for ck in range(CT):
                            nc.tensor.matmul(
                                ps[:, 0:C],
                                xt[:, ck, jt * 128 : (jt + 1) * 128],
                                w_sb[wn][:, ck, :],
                                start=(ck == 0),
                                stop=False,
                                skip_group_check=True,
                            )
                        nc.tensor.matmul(
                            ps[:, 0 : C + 2],
                            ones_sb[:, :],
                            bv_sb[bn][:, :],
                            start=False,
                            stop=True,
                            skip_group_check=True,
                        )
                        nc.vector.tensor_copy(dst[:, jt, :], ps[:, 0 : C + 2])

                for nm, xn, wn, bn in [
                    ("Q1", "x1", "wq1t", "bq1"),
                    ("K2", "x2", "wk2t", "bk2"),
                    ("Q2", "x2", "wq2t", "bq2"),
                    ("K1", "x1", "wk1t", "bk1"),
                ]:
                    t = pp.tile([128, CT, N], F32R, tag=nm)
                    proj_qk(t, x_sb[xn], wn, bn)
                    proj[nm] = t
                for nm, xn, wn, bn in [
                    ("VT2", "x2", "wv2t", "bv2"),
                    ("VT1", "x1", "wv1t", "bv1"),
                ]:
                    t = pp.tile([128, NT, C + 2], BF16, tag=nm)
                    proj_vt(t, x_sb[xn], wn, bn)
                    proj[nm] = t

            # ---------- attention, one direction at a time ----------
            with tc.tile_pool(name="ep", bufs=2) as ep:

                def exp_phase(Q, K, c0, cw):
                    e = ep.tile([128, NT, CW], BF16, tag="e")
                    for jt in range(NT):
                        ps = psp2.tile([128, CW], F32, tag="ps_s")
                        for ck in range(CT):
                            nc.tensor.matmul(
                                ps[:, 0:cw],
                                K[:, ck, jt * 128 : (jt + 1) * 128],
                                Q[:, ck, c0 : c0 + cw],
                                start=(ck == 0),
                                stop=(ck == CT - 1),
                            )
                        nc.scalar.activation(e[:, jt, 0:cw], ps[:, 0:cw], Exp)
                    return e

                def out_phase(e, VT, xt_dram, yt_dram, c0, cw):
                    for il in range(cw // 128):
                        it = c0 // 128 + il
                        po = psp.tile([128, C + 2], F32, tag="ps_o")
                        for jt in range(NT):
                            nc.tensor.matmul(
                                po[:, :],
                                e[:, jt, il * 128 : (il + 1) * 128],
                                VT[:, jt, :],
                                start=(jt == 0),
                                stop=(jt == NT - 1),
                            )
                        r = sp.tile([128, 1], F32, tag="r")
                        nc.vector.reciprocal(r[:, :], po[:, C : C + 1])
                        xt_t = sp.tile([128, C], F32, tag="xt")
                        nc.sync.dma_start(
                            out=xt_t[:, :], in_=xt_dram[it * 128 : (it + 1) * 128, :]
                        )
                        y = sp.tile([128, C], F32, tag="y")
                        nc.vector.tensor_scalar_mul(y[:, :], po[:, 0:C], r[:, :])
                        nc.vector.tensor_add(y[:, :], y[:, :], xt_t[:, :])
                        nc.sync.dma_start(
                            out=yt_dram[it * 128 : (it + 1) * 128, :], in_=y[:, :]
                        )

                # software pipeline across both directions: expS(k) || out(k-1)
                plan = [
                    (proj["Q1"], proj["K2"], proj["VT2"], xt_d["x1t"], y_d["y1t"], c0, cw)
                    for c0, cw in CHUNKS
                ] + [
                    (proj["Q2"], proj["K1"], proj["VT1"], xt_d["x2t"], y_d["y2t"], c0, cw)
                    for c0, cw in CHUNKS
                ]
                pending = None
                for Q, K, VT, xtd, ytd, c0, cw in plan:
                    e = exp_phase(Q, K, c0, cw)
                    if pending is not None:
                        out_phase(*pending)
                    pending = (e, VT, xtd, ytd, c0, cw)
                for args in [pending]:
                    out_phase(*args)

    nc.compile()
    return nc


def _get_nc():
    if "nc" not in _CACHE:
        _CACHE["nc"] = _build()
    return _CACHE["nc"]


def kernel(
    x1,
    x2,
    w_q1,
    b_q1,
    w_k1,
    b_k1,
    w_v1,
    b_v1,
    w_q2,
    b_q2,
    w_k2,
    b_k2,
    w_v2,
    b_v2,
    _trace=False,
):
    from concourse.bass_utils import run_bass_kernel_spmd

    nc = _get_nc()

    x1 = np.asarray(x1, dtype=np.float32)
    x2 = np.asarray(x2, dtype=np.float32)
    wT = {
        "wq1t": np.ascontiguousarray(np.asarray(w_q1, np.float32).T),
        "wk1t": np.ascontiguousarray(np.asarray(w_k1, np.float32).T),
        "wv1t": np.ascontiguousarray(np.asarray(w_v1, np.float32).T),
        "wq2t": np.ascontiguousarray(np.asarray(w_q2, np.float32).T),
        "wk2t": np.ascontiguousarray(np.asarray(w_k2, np.float32).T),
        "wv2t": np.ascontiguousarray(np.asarray(w_v2, np.float32).T),
    }
    bqk = {
        "bq1": np.asarray(b_q1, np.float32).reshape(C, 1),
        "bk1": np.asarray(b_k1, np.float32).reshape(C, 1),
        "bq2": np.asarray(b_q2, np.float32).reshape(C, 1),
        "bk2": np.asarray(b_k2, np.float32).reshape(C, 1),
    }
    bv = {
        "bv1": np.concatenate(
            [np.asarray(b_v1, np.float32).reshape(1, C), np.ones((1, 2), np.float32)], 1
        ),
        "bv2": np.concatenate(
            [np.asarray(b_v2, np.float32).reshape(1, C), np.ones((1, 2), np.float32)], 1
        ),
    }
    ones = np.ones((1, 128), np.float32)

    in_maps = []
    for i in range(B):
        x1i = np.ascontiguousarray(x1[i].reshape(C, N))
        x2i = np.ascontiguousarray(x2[i].reshape(C, N))
        m = {
            "x1": x1i,
            "x2": x2i,
            "x1t": np.ascontiguousarray(x1i.T),
            "x2t": np.ascontiguousarray(x2i.T),
            "ones": ones,
        }
        m.update(wT)
        m.update(bqk)
        m.update(bv)
        in_maps.append(m)

    res = run_bass_kernel_spmd(nc, in_maps, list(range(B)), trace=_trace)
    if _trace:
        _CACHE["last_result"] = res

    y1 = np.empty((B, C, H, W), np.float32)
    y2 = np.empty((B, C, H, W), np.float32)
    for i in range(B):
        y1[i] = res.results[i]["y1t"].T.reshape(C, H, W)
        y2[i] = res.results[i]["y2t"].T.reshape(C, H, W)
    return y1, y2


# revision 6
# speedup vs baseline: 1.2942x; 1.1655x over previous
"""Bidirectional cross-attention Trainium2 kernel.

Data-parallel over batch B=8 across 8 NeuronCores (1 sample/core).

Per-core dataflow (all matmuls f32r):
  Q1[c,n], K2[c,n]   : 1x1-conv projections, bias added per-partition on DVE
  V2T[n,c]+bias, plus a ones column at c=256 (bias folded via K=1 matmul)
  expS1T[j,i] = exp(K2^T Q1)  (ScalarE exp, chunked over i; no max-subtract,
                               logits are bounded for this problem's scale)
  outT[i, 0:257] = sum_j expS1T[j,i] * [V2T+b | 1][j, :]
     -> col 256 is the softmax denominator; y = outT[:,0:256]/denominator + x1T
  (symmetric for direction 2)

Host side: shard batch, transpose weights/x, gather + transpose outputs.
"""

import sys

if "/opt/trn_rl_repo" not in sys.path:
    sys.path.insert(0, "/opt/trn_rl_repo")

import numpy as np

B, C, H, W = 8, 256, 48, 48
N = H * W  # 2304
NT = N // 128  # 18 j/i tiles
CT = C // 128  # 2 c tiles
CW = 512  # max i-chunk width for expS (last chunk is 256)
CHUNKS = [(0, 512), (512, 512), (1024, 512), (1536, 512), (2048, 256)]

_CACHE = {}


def _build():
    import concourse.bacc as bacc
    import concourse.mybir as mybir
    from concourse.tile import TileContext

    F32, F16, BF16 = mybir.dt.float32, mybir.dt.float16, mybir.dt.bfloat16
    Exp = mybir.ActivationFunctionType.Exp

    nc = bacc.Bacc(None, target_bir_lowering=False)

    x_d = {
        "x1": nc.dram_tensor("x1", [C, N], F16, kind="ExternalInput"),
        "x2": nc.dram_tensor("x2", [C, N], F16, kind="ExternalInput"),
    }
    xt_d = {
        "x1t": nc.dram_tensor("x1t", [N, C], F32, kind="ExternalInput"),
        "x2t": nc.dram_tensor("x2t", [N, C], F32, kind="ExternalInput"),
    }
    w_names = ["wq1t", "wk1t", "wv1t", "wq2t", "wk2t", "wv2t"]
    w_d = {n: nc.dram_tensor(n, [C, C], F16, kind="ExternalInput") for n in w_names}
    bqk_names = ["bq1", "bk1", "bq2", "bk2"]
    bqk_d = {n: nc.dram_tensor(n, [C, 1], F32, kind="ExternalInput") for n in bqk_names}
    bv_d = {
        n: nc.dram_tensor(n, [1, C + 2], F16, kind="ExternalInput") for n in ["bv1", "bv2"]
    }
    ones_d = nc.dram_tensor("ones", [1, 128], F16, kind="ExternalInput")
    y_d = {
        "y1t": nc.dram_tensor("y1t", [N, C], F32, kind="ExternalOutput"),
        "y2t": nc.dram_tensor("y2t", [N, C], F32, kind="ExternalOutput"),
    }

    with TileContext(nc) as tc:
        with (
            tc.tile_pool(name="const", bufs=1) as cp,
            tc.tile_pool(name="proj", bufs=1) as pp,
            tc.tile_pool(name="stream", bufs=4) as sp,
            tc.tile_pool(name="psum", bufs=2, space="PSUM") as psp,
            tc.tile_pool(name="psum_s", bufs=4, space="PSUM") as psp2,
        ):
            # ---------- projections (x tiles freed after this block) ----------
            proj = {}
            with tc.tile_pool(name="xp", bufs=1) as xp:
                w_sb = {}

                def load_w(n):
                    t = cp.tile([128, CT, C], F16, tag=n)
                    for ck in range(CT):
                        nc.sync.dma_start(
                            out=t[:, ck, :], in_=w_d[n][ck * 128 : (ck + 1) * 128, :]
                        )
                    w_sb[n] = t

                # first projection's weights before the bulk x transfers
                for n in ["wq1t", "wk2t", "wv2t"]:
                    load_w(n)
                x_sb = {}
                for n in ["x1", "x2"]:
                    t = xp.tile([128, CT, N], F16, tag=n)
                    for ck in range(CT):
                        for h0, hw in [(0, 768), (768, 768), (1536, 768)]:
                            nc.sync.dma_start(
                                out=t[:, ck, h0 : h0 + hw],
                                in_=x_d[n][ck * 128 : (ck + 1) * 128, h0 : h0 + hw],
                            )
                    x_sb[n] = t
                for n in ["wq2t", "wk1t", "wv1t"]:
                    load_w(n)
                bqk_sb = {}
                for n in bqk_names:
                    t = cp.tile([128, CT], F32, tag=n)
                    for ct in range(CT):
                        nc.sync.dma_start(
                            out=t[:, ct : ct + 1], in_=bqk_d[n][ct * 128 : (ct + 1) * 128, :]
                        )
                    bqk_sb[n] = t
                bv_sb = {}
                for n in ["bv1", "bv2"]:
                    t = cp.tile([1, C + 2], F16, tag=n)
                    nc.sync.dma_start(out=t[:, :], in_=bv_d[n][:, :])
                    bv_sb[n] = t
                ones_sb = cp.tile([1, 128], F16, tag="ones")
                nc.sync.dma_start(out=ones_sb[:, :], in_=ones_d[:, :])

                def proj_qk(dst, xt, wn, bn):
                    for ct in range(CT):
                        for c0, cw in CHUNKS:
                            ps = psp.tile([128, CW], F32, tag="pj")
                            for ck in range(CT):
                                nc.tensor.matmul(
                                    ps[:, 0:cw],
                                    w_sb[wn][:, ck, ct * 128 : (ct + 1) * 128],
                                    xt[:, ck, c0 : c0 + cw],
                                    start=(ck == 0),
                                    stop=(ck == CT - 1),
                                )
                            nc.vector.tensor_scalar_add(
                                dst[:, ct, c0 : c0 + cw],
                                ps[:, 0:cw],
                                bqk_sb[bn][:, ct : ct + 1],
                            )

                def proj_vt(dst, xt, wn, bn):
                    for jt in range(NT):
                        ps = psp.tile([128, CW], F32, tag="pj")
                        for ck in range(CT):
                            nc.tensor.matmul(
                                ps[:, 0:C],
                                xt[:, ck, jt * 128 : (jt + 1) * 128],
                                w_sb[wn][:, ck, :],
                                start=(ck == 0),
                                stop=False,
                                skip_group_check=True,
                            )
                        nc.tensor.matmul(
                            ps[:, 0 : C + 2],
                            ones_sb[:, :],
                            bv_sb[bn][:, :],
                            start=False,
                            stop=True,
                            skip_group_check=True,
                        )
                        nc.vector.tensor_copy(dst[:, jt, :], ps[:, 0 : C + 2])

                for nm, xn, wn, bn in [
                    ("Q1", "x1", "wq1t", "bq1"),
                    ("K2", "x2", "wk2t", "bk2"),
                    ("Q2", "x2", "wq2t", "bq2"),
                    ("K1", "x1", "wk1t", "bk1"),
                ]:
                    t = pp.tile([128, CT, N], F16, tag=nm)
                    proj_qk(t, x_sb[xn], wn, bn)
                    proj[nm] = t
                for nm, xn, wn, bn in [
                    ("VT2", "x2", "wv2t", "bv2"),
                    ("VT1", "x1", "wv1t", "bv1"),
                ]:
                    t = pp.tile([128, NT, C + 2], BF16, tag=nm)
                    proj_vt(t, x_sb[xn], wn, bn)
                    proj[nm] = t

            # ---------- attention, one direction at a time ----------
            with tc.tile_pool(name="ep", bufs=2) as ep:

                def exp_phase(Q, K, c0, cw):
                    e = ep.tile([128, NT, CW], BF16, tag="e")
                    for jt in range(NT):
                        ps = psp2.tile([128, CW], F32, tag="ps_s")
                        for ck in range(CT):
                            nc.tensor.matmul(
                                ps[:, 0:cw],
                                K[:, ck, jt * 128 : (jt + 1) * 128],
                                Q[:, ck, c0 : c0 + cw],
                                start=(ck == 0),
                                stop=(ck == CT - 1),
                            )
                        nc.scalar.activation(e[:, jt, 0:cw], ps[:, 0:cw], Exp)
                    return e

                def out_phase(e, VT, xt_dram, yt_dram, c0, cw):
                    for il in range(cw // 128):
                        it = c0 // 128 + il
                        po = psp.tile([128, C + 2], F32, tag="ps_o")
                        for jt in range(NT):
                            nc.tensor.matmul(
                                po[:, :],
                                e[:, jt, il * 128 : (il + 1) * 128],
                                VT[:, jt, :],
                                start=(jt == 0),
                                stop=(jt == NT - 1),
                            )
                        r = sp.tile([128, 1], F32, tag="r")
                        nc.vector.reciprocal(r[:, :], po[:, C : C + 1])
                        xt_t = sp.tile([128, C], F32, tag="xt")
                        nc.sync.dma_start(
                            out=xt_t[:, :], in_=xt_dram[it * 128 : (it + 1) * 128, :]
                        )
                        y = sp.tile([128, C], F32, tag="y")
                        nc.vector.tensor_scalar_mul(y[:, :], po[:, 0:C], r[:, :])
                        nc.vector.tensor_add(y[:, :], y[:, :], xt_t[:, :])
                        nc.sync.dma_start(
                            out=yt_dram[it * 128 : (it + 1) * 128, :], in_=y[:, :]
                        )

                # software pipeline across both directions: expS(k) || out(k-1)
                plan = [
                    (proj["Q1"], proj["K2"], proj["VT2"], xt_d["x1t"], y_d["y1t"], c0, cw)
                    for c0, cw in CHUNKS
                ] + [
                    (proj["Q2"], proj["K1"], proj["VT1"], xt_d["x2t"], y_d["y2t"], c0, cw)
                    for c0, cw in CHUNKS
                ]
                pending = None
                for Q, K, VT, xtd, ytd, c0, cw in plan:
                    e = exp_phase(Q, K, c0, cw)
                    if pending is not None:
                        out_phase(*pending)
                    pending = (e, VT, xtd, ytd, c0, cw)
                for args in [pending]:
                    out_phase(*args)

    nc.compile()
    return nc


def _get_nc():
    if "nc" not in _CACHE:
        _CACHE["nc"] = _build()
    return _CACHE["nc"]


def kernel(
    x1,
    x2,
    w_q1,
    b_q1,
    w_k1,
    b_k1,
    w_v1,
    b_v1,
    w_q2,
    b_q2,
    w_k2,
    b_k2,
    w_v2,
    b_v2,
    _trace=False,
):
    from concourse.bass_utils import run_bass_kernel_spmd

    nc = _get_nc()

    x1 = np.asarray(x1, dtype=np.float32)
    x2 = np.asarray(x2, dtype=np.float32)
    x1h = x1.astype(np.float16)
    x2h = x2.astype(np.float16)
    wT = {
        "wq1t": np.ascontiguousarray(np.asarray(w_q1, np.float32).T.astype(np.float16)),
        "wk1t": np.ascontiguousarray(np.asarray(w_k1, np.float32).T.astype(np.float16)),
        "wv1t": np.ascontiguousarray(np.asarray(w_v1, np.float32).T.astype(np.float16)),
        "wq2t": np.ascontiguousarray(np.asarray(w_q2, np.float32).T.astype(np.float16)),
        "wk2t": np.ascontiguousarray(np.asarray(w_k2, np.float32).T.astype(np.float16)),
        "wv2t": np.ascontiguousarray(np.asarray(w_v2, np.float32).T.astype(np.float16)),
    }
    bqk = {
        "bq1": np.asarray(b_q1, np.float32).reshape(C, 1),
        "bk1": np.asarray(b_k1, np.float32).reshape(C, 1),
        "bq2": np.asarray(b_q2, np.float32).reshape(C, 1),
        "bk2": np.asarray(b_k2, np.float32).reshape(C, 1),
    }
    bv = {
        "bv1": np.concatenate(
            [np.asarray(b_v1, np.float32).reshape(1, C), np.ones((1, 2), np.float32)], 1
        ).astype(np.float16),
        "bv2": np.concatenate(
            [np.asarray(b_v2, np.float32).reshape(1, C), np.ones((1, 2), np.float32)], 1
        ).astype(np.float16),
    }
    ones = np.ones((1, 128), np.float16)

    in_maps = []
    for i in range(B):
        x1i = np.ascontiguousarray(x1[i].reshape(C, N))
        x2i = np.ascontiguousarray(x2[i].reshape(C, N))
        m = {
            "x1": np.ascontiguousarray(x1h[i].reshape(C, N)),
            "x2": np.ascontiguousarray(x2h[i].reshape(C, N)),
            "x1t": np.ascontiguousarray(x1i.T),
            "x2t": np.ascontiguousarray(x2i.T),
            "ones": ones,
        }
        m.update(wT)
        m.update(bqk)
        m.update(bv)
        in_maps.append(m)

    res = run_bass_kernel_spmd(nc, in_maps, list(range(B)), trace=_trace)
    if _trace:
        _CACHE["last_result"] = res

    y1 = np.empty((B, C, H, W), np.float32)
    y2 = np.empty((B, C, H, W), np.float32)
    for i in range(B):
        y1[i] = res.results[i]["y1t"].T.reshape(C, H, W)
        y2[i] = res.results[i]["y2t"].T.reshape(C, H, W)
    return y1, y2


# revision 9
# speedup vs baseline: 1.3122x; 1.0139x over previous
"""Bidirectional cross-attention Trainium2 kernel.

Data-parallel over batch B=8 across 8 NeuronCores (1 sample/core).

Per-core dataflow (all matmuls f32r):
  Q1[c,n], K2[c,n]   : 1x1-conv projections, bias added per-partition on DVE
  V2T[n,c]+bias, plus a ones column at c=256 (bias folded via K=1 matmul)
  expS1T[j,i] = exp(K2^T Q1)  (ScalarE exp, chunked over i; no max-subtract,
                               logits are bounded for this problem's scale)
  outT[i, 0:257] = sum_j expS1T[j,i] * [V2T+b | 1][j, :]
     -> col 256 is the softmax denominator; y = outT[:,0:256]/denominator + x1T
  (symmetric for direction 2)

Host side: shard batch, transpose weights/x, gather + transpose outputs.
"""

import sys

if "/opt/trn_rl_repo" not in sys.path:
    sys.path.insert(0, "/opt/trn_rl_repo")

import numpy as np

B, C, H, W = 8, 256, 48, 48
N = H * W  # 2304
NT = N // 128  # 18 j/i tiles
CT = C // 128  # 2 c tiles
CW = 512  # max i-chunk width for expS (last chunk is 256)
CHUNKS = [(0, 512), (512, 512), (1024, 512), (1536, 512), (2048, 256)]

_CACHE = {}


def _build():
    import concourse.bacc as bacc
    import concourse.mybir as mybir
    from concourse.tile import TileContext

    F32, F16, BF16 = mybir.dt.float32, mybir.dt.float16, mybir.dt.bfloat16
    Exp = mybir.ActivationFunctionType.Exp

    nc = bacc.Bacc(None, target_bir_lowering=False)

    x_d = {
        "x1": nc.dram_tensor("x1", [C, N], F16, kind="ExternalInput"),
        "x2": nc.dram_tensor("x2", [C, N], F16, kind="ExternalInput"),
    }
    xt_d = {
        "x1t": nc.dram_tensor("x1t", [N, C], F32, kind="ExternalInput"),
        "x2t": nc.dram_tensor("x2t", [N, C], F32, kind="ExternalInput"),
    }
    w_names = ["wq1t", "wk1t", "wv1t", "wq2t", "wk2t", "wv2t"]
    w_d = {n: nc.dram_tensor(n, [C, C], F16, kind="ExternalInput") for n in w_names}
    bqk_names = ["bq1", "bk1", "bq2", "bk2"]
    bqk_d = {n: nc.dram_tensor(n, [C, 1], F32, kind="ExternalInput") for n in bqk_names}
    bv_d = {
        n: nc.dram_tensor(n, [1, C + 2], F16, kind="ExternalInput") for n in ["bv1", "bv2"]
    }
    ones_d = nc.dram_tensor("ones", [1, 128], F16, kind="ExternalInput")
    y_d = {
        "y1t": nc.dram_tensor("y1t", [N, C], F32, kind="ExternalOutput"),
        "y2t": nc.dram_tensor("y2t", [N, C], F32, kind="ExternalOutput"),
    }

    with TileContext(nc) as tc:
        with (
            tc.tile_pool(name="const", bufs=1) as cp,
            tc.tile_pool(name="proj", bufs=1) as pp,
            tc.tile_pool(name="stream", bufs=4) as sp,
            tc.tile_pool(name="psum", bufs=2, space="PSUM") as psp,
            tc.tile_pool(name="psum_s", bufs=4, space="PSUM") as psp2,
        ):
            # ---------- projections (x tiles freed after this block) ----------
            proj = {}
            with tc.tile_pool(name="xp", bufs=1) as xp:
                w_sb = {}

                def load_w(n):
                    t = cp.tile([128, CT, C], F16, tag=n)
                    for ck in range(CT):
                        nc.sync.dma_start(
                            out=t[:, ck, :], in_=w_d[n][ck * 128 : (ck + 1) * 128, :]
                        )
                    w_sb[n] = t

                # first projection's weights + small constants before bulk x
                for n in ["wq1t", "wk2t", "wv2t"]:
                    load_w(n)
                bqk_sb = {}
                for n in bqk_names:
                    t = cp.tile([128, CT], F32, tag=n)
                    for ct in range(CT):
                        nc.sync.dma_start(
                            out=t[:, ct : ct + 1], in_=bqk_d[n][ct * 128 : (ct + 1) * 128, :]
                        )
                    bqk_sb[n] = t
                bv_sb = {}
                for n in ["bv1", "bv2"]:
                    t = cp.tile([1, C + 2], F16, tag=n)
                    nc.sync.dma_start(out=t[:, :], in_=bv_d[n][:, :])
                    bv_sb[n] = t
                ones_sb = cp.tile([1, 128], F16, tag="ones")
                nc.sync.dma_start(out=ones_sb[:, :], in_=ones_d[:, :])
                x_sb = {}
                for n in ["x1", "x2"]:
                    t = xp.tile([128, CT, N], F16, tag=n)
                    for ck in range(CT):
                        for h0, hw in [(0, 768), (768, 768), (1536, 768)]:
                            nc.sync.dma_start(
                                out=t[:, ck, h0 : h0 + hw],
                                in_=x_d[n][ck * 128 : (ck + 1) * 128, h0 : h0 + hw],
                            )
                    x_sb[n] = t
                for n in ["wq2t", "wk1t", "wv1t"]:
                    load_w(n)

                def proj_qk(dst, xt, wn, bn):
                    for ct in range(CT):
                        for c0, cw in CHUNKS:
                            ps = psp.tile([128, CW], F32, tag="pj")
                            for ck in range(CT):
                                nc.tensor.matmul(
                                    ps[:, 0:cw],
                                    w_sb[wn][:, ck, ct * 128 : (ct + 1) * 128],
                                    xt[:, ck, c0 : c0 + cw],
                                    start=(ck == 0),
                                    stop=(ck == CT - 1),
                                )
                            nc.vector.tensor_scalar_add(
                                dst[:, ct, c0 : c0 + cw],
                                ps[:, 0:cw],
                                bqk_sb[bn][:, ct : ct + 1],
                            )

                def proj_vt(dst, xt, wn, bn):
                    for jt in range(NT):
                        ps = psp.tile([128, CW], F32, tag="pj")
                        for ck in range(CT):
                            nc.tensor.matmul(
                                ps[:, 0:C],
                                xt[:, ck, jt * 128 : (jt + 1) * 128],
                                w_sb[wn][:, ck, :],
                                start=(ck == 0),
                                stop=False,
                                skip_group_check=True,
                            )
                        nc.tensor.matmul(
                            ps[:, 0 : C + 2],
                            ones_sb[:, :],
                            bv_sb[bn][:, :],
                            start=False,
                            stop=True,
                            skip_group_check=True,
                        )
                        nc.vector.tensor_copy(dst[:, jt, :], ps[:, 0 : C + 2])

                for nm, xn, wn, bn in [
                    ("Q1", "x1", "wq1t", "bq1"),
                    ("K2", "x2", "wk2t", "bk2"),
                    ("Q2", "x2", "wq2t", "bq2"),
                    ("K1", "x1", "wk1t", "bk1"),
                ]:
                    t = pp.tile([128, CT, N], F16, tag=nm)
                    proj_qk(t, x_sb[xn], wn, bn)
                    proj[nm] = t
                for nm, xn, wn, bn in [
                    ("VT2", "x2", "wv2t", "bv2"),
                    ("VT1", "x1", "wv1t", "bv1"),
                ]:
                    t = pp.tile([128, NT, C + 2], BF16, tag=nm)
                    proj_vt(t, x_sb[xn], wn, bn)
                    proj[nm] = t

            # ---------- attention, one direction at a time ----------
            with tc.tile_pool(name="ep", bufs=2) as ep:

                def exp_actions(Q, K, e, c0, cw):
                    # one action = expS matmul pair + exp for one j-tile
                    def mk(jt):
                        def act():
                            ps = psp2.tile([128, CW], F32, tag="ps_s")
                            for ck in range(CT):
                                nc.tensor.matmul(
                                    ps[:, 0:cw],
                                    K[:, ck, jt * 128 : (jt + 1) * 128],
                                    Q[:, ck, c0 : c0 + cw],
                                    start=(ck == 0),
                                    stop=(ck == CT - 1),
                                )
                            nc.scalar.activation(e[:, jt, 0:cw], ps[:, 0:cw], Exp)

                        return act

                    return [mk(jt) for jt in range(NT)]

                def out_actions(e, VT, xt_dram, yt_dram, c0, cw):
                    # actions = out-matmul slices + epilogue, per i-subtile
                    acts = []
                    for il in range(cw // 128):
                        it = c0 // 128 + il
                        po = psp.tile([128, C + 2], F32, tag="ps_o")

                        def mk_mm(po, il, j0, jn):
                            def act():
                                for jt in range(j0, jn):
                                    nc.tensor.matmul(
                                        po[:, :],
                                        e[:, jt, il * 128 : (il + 1) * 128],
                                        VT[:, jt, :],
                                        start=(jt == 0),
                                        stop=(jt == NT - 1),
                                    )

                            return act

                        for j0 in range(0, NT, 5):
                            acts.append(mk_mm(po, il, j0, min(j0 + 5, NT)))

                        def mk_epi(po, it):
                            def act():
                                r = sp.tile([128, 1], F32, tag="r")
                                nc.vector.reciprocal(r[:, :], po[:, C : C + 1])
                                xt_t = sp.tile([128, C], F32, tag="xt")
                                nc.sync.dma_start(
                                    out=xt_t[:, :],
                                    in_=xt_dram[it * 128 : (it + 1) * 128, :],
                                )
                                y = sp.tile([128, C], F32, tag="y")
                                nc.vector.tensor_scalar_mul(y[:, :], po[:, 0:C], r[:, :])
                                nc.vector.tensor_add(y[:, :], y[:, :], xt_t[:, :])
                                nc.sync.dma_start(
                                    out=yt_dram[it * 128 : (it + 1) * 128, :], in_=y[:, :]
                                )

                            return act

                        acts.append(mk_epi(po, it))
                    return acts

                def weave(a, b):
                    # emit all of a and b interleaved evenly (a paces, b fills)
                    if not b:
                        for f in a:
                            f()
                        return
                    na, nb = len(a), len(b)
                    j = 0
                    for i, f in enumerate(a):
                        f()
                        while j < nb and j * na <= (i + 1) * nb - 1:
                            b[j]()
                            j += 1
                    while j < nb:
                        b[j]()
                        j += 1

                # software pipeline across both directions: expS(k) woven with out(k-1)
                plan = [
                    (proj["Q1"], proj["K2"], proj["VT2"], xt_d["x1t"], y_d["y1t"], c0, cw)
                    for c0, cw in CHUNKS
                ] + [
                    (proj["Q2"], proj["K1"], proj["VT1"], xt_d["x2t"], y_d["y2t"], c0, cw)
                    for c0, cw in CHUNKS
                ]
                pending = []
                for Q, K, VT, xtd, ytd, c0, cw in plan:
                    e = ep.tile([128, NT, CW], BF16, tag="e")
                    weave(exp_actions(Q, K, e, c0, cw), pending)
                    pending = out_actions(e, VT, xtd, ytd, c0, cw)
                weave(pending, [])

    nc.compile()
    return nc


def _get_nc():
    if "nc" not in _CACHE:
        _CACHE["nc"] = _build()
    return _CACHE["nc"]


def kernel(
    x1,
    x2,
    w_q1,
    b_q1,
    w_k1,
    b_k1,
    w_v1,
    b_v1,
    w_q2,
    b_q2,
    w_k2,
    b_k2,
    w_v2,
    b_v2,
    _trace=False,
):
    from concourse.bass_utils import run_bass_kernel_spmd

    nc = _get_nc()

    x1 = np.asarray(x1, dtype=np.float32)
    x2 = np.asarray(x2, dtype=np.float32)
    x1h = x1.astype(np.float16)
    x2h = x2.astype(np.float16)
    wT = {
        "wq1t": np.ascontiguousarray(np.asarray(w_q1, np.float32).T.astype(np.float16)),
        "wk1t": np.ascontiguousarray(np.asarray(w_k1, np.float32).T.astype(np.float16)),
        "wv1t": np.ascontiguousarray(np.asarray(w_v1, np.float32).T.astype(np.float16)),
        "wq2t": np.ascontiguousarray(np.asarray(w_q2, np.float32).T.astype(np.float16)),
        "wk2t": np.ascontiguousarray(np.asarray(w_k2, np.float32).T.astype(np.float16)),
        "wv2t": np.ascontiguousarray(np.asarray(w_v2, np.float32).T.astype(np.float16)),
    }
    bqk = {
        "bq1": np.asarray(b_q1, np.float32).reshape(C, 1),
        "bk1": np.asarray(b_k1, np.float32).reshape(C, 1),
        "bq2": np.asarray(b_q2, np.float32).reshape(C, 1),
        "bk2": np.asarray(b_k2, np.float32).reshape(C, 1),
    }
    bv = {
        "bv1": np.concatenate(
            [np.asarray(b_v1, np.float32).reshape(1, C), np.ones((1, 2), np.float32)], 1
        ).astype(np.float16),
        "bv2": np.concatenate(
            [np.asarray(b_v2, np.float32).reshape(1, C), np.ones((1, 2), np.float32)], 1
        ).astype(np.float16),
    }
    ones = np.ones((1, 128), np.float16)

    in_maps = []
    for i in range(B):
        x1i = np.ascontiguousarray(x1[i].reshape(C, N))
        x2i = np.ascontiguousarray(x2[i].reshape(C, N))
        m = {
            "x1": np.ascontiguousarray(x1h[i].reshape(C, N)),
            "x2": np.ascontiguousarray(x2h[i].reshape(C, N)),
            "x1t": np.ascontiguousarray(x1i.T),
            "x2t": np.ascontiguousarray(x2i.T),
            "ones": ones,
        }
        m.update(wT)
        m.update(bqk)
        m.update(bv)
        in_maps.append(m)

    res = run_bass_kernel_spmd(nc, in_maps, list(range(B)), trace=_trace)
    if _trace:
        _CACHE["last_result"] = res

    y1 = np.empty((B, C, H, W), np.float32)
    y2 = np.empty((B, C, H, W), np.float32)
    for i in range(B):
        y1[i] = res.results[i]["y1t"].T.reshape(C, H, W)
        y2[i] = res.results[i]["y2t"].T.reshape(C, H, W)
    return y1, y2


# revision 11
# speedup vs baseline: 1.3649x; 1.0402x over previous
"""Bidirectional cross-attention Trainium2 kernel.

Data-parallel over batch B=8 across 8 NeuronCores (1 sample/core).

Per-core dataflow (all matmuls f32r):
  Q1[c,n], K2[c,n]   : 1x1-conv projections, bias added per-partition on DVE
  V2T[n,c]+bias, plus a ones column at c=256 (bias folded via K=1 matmul)
  expS1T[j,i] = exp(K2^T Q1)  (ScalarE exp, chunked over i; no max-subtract,
                               logits are bounded for this problem's scale)
  outT[i, 0:257] = sum_j expS1T[j,i] * [V2T+b | 1][j, :]
     -> col 256 is the softmax denominator; y = outT[:,0:256]/denominator + x1T
  (symmetric for direction 2)

Host side: shard batch, transpose weights/x, gather + transpose outputs.
"""

import sys

if "/opt/trn_rl_repo" not in sys.path:
    sys.path.insert(0, "/opt/trn_rl_repo")

import numpy as np

B, C, H, W = 8, 256, 48, 48
N = H * W  # 2304
NT = N // 128  # 18 j/i tiles
CT = C // 128  # 2 c tiles
CW = 512  # max i-chunk width for expS (last chunk is 256)
CHUNKS = [(0, 512), (512, 512), (1024, 512), (1536, 512), (2048, 256)]

_CACHE = {}


def _build():
    import concourse.bacc as bacc
    import concourse.mybir as mybir
    from concourse.tile import TileContext

    F32, F16, BF16 = mybir.dt.float32, mybir.dt.float16, mybir.dt.bfloat16
    Exp = mybir.ActivationFunctionType.Exp
    Ident = mybir.ActivationFunctionType.Identity

    nc = bacc.Bacc(None, target_bir_lowering=False)

    x_d = {
        "x1": nc.dram_tensor("x1", [C, N], F16, kind="ExternalInput"),
        "x2": nc.dram_tensor("x2", [C, N], F16, kind="ExternalInput"),
    }
    xt_d = {
        "x1t": nc.dram_tensor("x1t", [N, C], F32, kind="ExternalInput"),
        "x2t": nc.dram_tensor("x2t", [N, C], F32, kind="ExternalInput"),
    }
    w_names = ["wq1t", "wk2t", "wv2t", "wq2t", "wk1t", "wv1t"]  # pack order
    wpack_d = nc.dram_tensor("wpack", [C, 6 * C], F16, kind="ExternalInput")
    bqk_names = ["bq1", "bk1", "bq2", "bk2"]  # pack order
    bqk_d = nc.dram_tensor("bqk", [C, 4], F32, kind="ExternalInput")
    # bvpack: [bv1(258) | bv2(258) | ones(128)]
    bvpack_d = nc.dram_tensor("bvpack", [1, 644], F16, kind="ExternalInput")
    y_d = {
        "y1t": nc.dram_tensor("y1t", [N, C], F32, kind="ExternalOutput"),
        "y2t": nc.dram_tensor("y2t", [N, C], F32, kind="ExternalOutput"),
    }

    with TileContext(nc) as tc:
        with (
            tc.tile_pool(name="const", bufs=1) as cp,
            tc.tile_pool(name="proj", bufs=1) as pp,
            tc.tile_pool(name="stream", bufs=4) as sp,
            tc.tile_pool(name="psum", bufs=2, space="PSUM") as psp,
            tc.tile_pool(name="psum_s", bufs=4, space="PSUM") as psp2,
        ):
            # ---------- projections (x tiles freed after this block) ----------
            proj = {}
            with tc.tile_pool(name="xp", bufs=1) as xp:
                # x1 first (feeds the first projection), then packed consts, then x2
                x_sb = {}

                def load_x(n):
                    t = xp.tile([128, CT, N], F16, tag=n)
                    for ck in range(CT):
                        nc.sync.dma_start(
                            out=t[:, ck, :], in_=x_d[n][ck * 128 : (ck + 1) * 128, :]
                        )
                    x_sb[n] = t

                load_x("x1")
                wpack = cp.tile([128, CT, 6 * C], F16, tag="wpack")
                for ck in range(CT):
                    nc.sync.dma_start(
                        out=wpack[:, ck, :], in_=wpack_d[ck * 128 : (ck + 1) * 128, :]
                    )
                w_sb = {
                    n: wpack[:, :, i * C : (i + 1) * C] for i, n in enumerate(w_names)
                }
                bqkt = cp.tile([128, CT, 4], F32, tag="bqkt")
                for ck in range(CT):
                    nc.sync.dma_start(
                        out=bqkt[:, ck, :], in_=bqk_d[ck * 128 : (ck + 1) * 128, :]
                    )
                bqk_sb = {n: bqkt[:, :, i] for i, n in enumerate(bqk_names)}
                bvpack = cp.tile([1, 644], F16, tag="bvpack")
                nc.sync.dma_start(out=bvpack[:, :], in_=bvpack_d[:, :])
                bv_sb = {"bv1": bvpack[:, 0:258], "bv2": bvpack[:, 258:516]}
                ones_sb = bvpack[:, 516:644]
                load_x("x2")

                def proj_qk(dst, xt, wn, bn):
                    for ct in range(CT):
                        for c0, cw in CHUNKS:
                            ps = psp.tile([128, CW], F32, tag="pj")
                            for ck in range(CT):
                                nc.tensor.matmul(
                                    ps[:, 0:cw],
                                    w_sb[wn][:, ck, ct * 128 : (ct + 1) * 128],
                                    xt[:, ck, c0 : c0 + cw],
                                    start=(ck == 0),
                                    stop=(ck == CT - 1),
                                )
                            nc.scalar.activation(
                                dst[:, ct, c0 : c0 + cw],
                                ps[:, 0:cw],
                                Ident,
                                bias=bqk_sb[bn][:, ct : ct + 1],
                            )

                def proj_vt(dst, xt, wn, bn):
                    for jt in range(NT):
                        ps = psp.tile([128, CW], F32, tag="pj")
                        for ck in range(CT):
                            nc.tensor.matmul(
                                ps[:, 0:C],
                                xt[:, ck, jt * 128 : (jt + 1) * 128],
                                w_sb[wn][:, ck, :],
                                start=(ck == 0),
                                stop=False,
                                skip_group_check=True,
                            )
                        nc.tensor.matmul(
                            ps[:, 0 : C + 2],
                            ones_sb,
                            bv_sb[bn],
                            start=False,
                            stop=True,
                            skip_group_check=True,
                        )
                        nc.vector.tensor_copy(dst[:, jt, :], ps[:, 0 : C + 2])

                for nm, xn, wn, bn in [
                    ("Q1", "x1", "wq1t", "bq1"),
                    ("K2", "x2", "wk2t", "bk2"),
                    ("Q2", "x2", "wq2t", "bq2"),
                    ("K1", "x1", "wk1t", "bk1"),
                ]:
                    t = pp.tile([128, CT, N], F16, tag=nm)
                    proj_qk(t, x_sb[xn], wn, bn)
                    proj[nm] = t
                for nm, xn, wn, bn in [
                    ("VT2", "x2", "wv2t", "bv2"),
                    ("VT1", "x1", "wv1t", "bv1"),
                ]:
                    t = pp.tile([128, NT, C + 2], BF16, tag=nm)
                    proj_vt(t, x_sb[xn], wn, bn)
                    proj[nm] = t

            # ---------- attention, one direction at a time ----------
            with tc.tile_pool(name="ep", bufs=2) as ep:

                def exp_actions(Q, K, e, c0, cw):
                    # one action = expS matmul pair + exp for one j-tile
                    def mk(jt):
                        def act():
                            ps = psp2.tile([128, CW], F32, tag="ps_s")
                            for ck in range(CT):
                                nc.tensor.matmul(
                                    ps[:, 0:cw],
                                    K[:, ck, jt * 128 : (jt + 1) * 128],
                                    Q[:, ck, c0 : c0 + cw],
                                    start=(ck == 0),
                                    stop=(ck == CT - 1),
                                )
                            nc.scalar.activation(e[:, jt, 0:cw], ps[:, 0:cw], Exp)

                        return act

                    return [mk(jt) for jt in range(NT)]

                def out_actions(e, VT, xt_dram, yt_dram, c0, cw):
                    # actions = out-matmul slices + epilogue, per i-subtile
                    acts = []
                    for il in range(cw // 128):
                        it = c0 // 128 + il
                        po = psp.tile([128, C + 2], F32, tag="ps_o")

                        xt_t = sp.tile([128, C], F32, tag="xt")

                        def mk_mm(po, il, it, j0, jn, xt_t):
                            def act():
                                if j0 == 0:
                                    nc.sync.dma_start(
                                        out=xt_t[:, :],
                                        in_=xt_dram[it * 128 : (it + 1) * 128, :],
                                    )
                                for jt in range(j0, jn):
                                    nc.tensor.matmul(
                                        po[:, :],
                                        e[:, jt, il * 128 : (il + 1) * 128],
                                        VT[:, jt, :],
                                        start=(jt == 0),
                                        stop=(jt == NT - 1),
                                    )

                            return act

                        for j0 in range(0, NT, 5):
                            acts.append(mk_mm(po, il, it, j0, min(j0 + 5, NT), xt_t))

                        def mk_epi(po, it, xt_t):
                            def act():
                                r = sp.tile([128, 1], F32, tag="r")
                                nc.vector.reciprocal(r[:, :], po[:, C : C + 1])
                                y = sp.tile([128, C], F32, tag="y")
                                nc.vector.scalar_tensor_tensor(
                                    y[:, :],
                                    po[:, 0:C],
                                    r[:, :],
                                    xt_t[:, :],
                                    op0=mybir.AluOpType.mult,
                                    op1=mybir.AluOpType.add,
                                )
                                nc.sync.dma_start(
                                    out=yt_dram[it * 128 : (it + 1) * 128, :], in_=y[:, :]
                                )

                            return act

                        acts.append(mk_epi(po, it, xt_t))
                    return acts

                def weave(a, b):
                    # emit all of a and b interleaved evenly (a paces, b fills)
                    if not b:
                        for f in a:
                            f()
                        return
                    na, nb = len(a), len(b)
                    j = 0
                    for i, f in enumerate(a):
                        f()
                        while j < nb and j * na <= (i + 1) * nb - 1:
                            b[j]()
                            j += 1
                    while j < nb:
                        b[j]()
                        j += 1

                # software pipeline across both directions: expS(k) woven with out(k-1)
                plan = [
                    (proj["Q1"], proj["K2"], proj["VT2"], xt_d["x1t"], y_d["y1t"], c0, cw)
                    for c0, cw in CHUNKS
                ] + [
                    (proj["Q2"], proj["K1"], proj["VT1"], xt_d["x2t"], y_d["y2t"], c0, cw)
                    for c0, cw in CHUNKS
                ]
                pending = []
                for Q, K, VT, xtd, ytd, c0, cw in plan:
                    e = ep.tile([128, NT, CW], BF16, tag="e")
                    weave(exp_actions(Q, K, e, c0, cw), pending)
                    pending = out_actions(e, VT, xtd, ytd, c0, cw)
                weave(pending, [])

    nc.compile()
    return nc


def _get_nc():
    if "nc" not in _CACHE:
        _CACHE["nc"] = _build()
    return _CACHE["nc"]


def kernel(
    x1,
    x2,
    w_q1,
    b_q1,
    w_k1,
    b_k1,
    w_v1,
    b_v1,
    w_q2,
    b_q2,
    w_k2,
    b_k2,
    w_v2,
    b_v2,
    _trace=False,
):
    from concourse.bass_utils import run_bass_kernel_spmd

    nc = _get_nc()

    x1 = np.asarray(x1, dtype=np.float32)
    x2 = np.asarray(x2, dtype=np.float32)
    x1h = x1.astype(np.float16)
    x2h = x2.astype(np.float16)
    # wpack order must match w_names: wq1t, wk2t, wv2t, wq2t, wk1t, wv1t
    wpack = np.ascontiguousarray(
        np.concatenate(
            [np.asarray(w, np.float32).T for w in [w_q1, w_k2, w_v2, w_q2, w_k1, w_v1]],
            axis=1,
        ).astype(np.float16)
    )
    bqk = np.ascontiguousarray(
        np.stack(
            [np.asarray(b, np.float32) for b in [b_q1, b_k1, b_q2, b_k2]], axis=1
        )
    )
    bv1 = np.concatenate(
        [np.asarray(b_v1, np.float32).reshape(1, C), np.ones((1, 2), np.float32)], 1
    )
    bv2 = np.concatenate(
        [np.asarray(b_v2, np.float32).reshape(1, C), np.ones((1, 2), np.float32)], 1
    )
    bvpack = np.concatenate([bv1, bv2, np.ones((1, 128), np.float32)], 1).astype(
        np.float16
    )

    in_maps = []
    for i in range(B):
        x1i = np.ascontiguousarray(x1[i].reshape(C, N))
        x2i = np.ascontiguousarray(x2[i].reshape(C, N))
        m = {
            "x1": np.ascontiguousarray(x1h[i].reshape(C, N)),
            "x2": np.ascontiguousarray(x2h[i].reshape(C, N)),
            "x1t": np.ascontiguousarray(x1i.T),
            "x2t": np.ascontiguousarray(x2i.T),
            "wpack": wpack,
            "bqk": bqk,
            "bvpack": bvpack,
        }
        in_maps.append(m)

    res = run_bass_kernel_spmd(nc, in_maps, list(range(B)), trace=_trace)
    if _trace:
        _CACHE["last_result"] = res

    y1 = np.empty((B, C, H, W), np.float32)
    y2 = np.empty((B, C, H, W), np.float32)
    for i in range(B):
        y1[i] = res.results[i]["y1t"].T.reshape(C, H, W)
        y2[i] = res.results[i]["y2t"].T.reshape(C, H, W)
    return y1, y2
